# revision 1
# baseline (speedup 1.0000x reference)
"""Trainium2 Bass kernel for nn_DividedSsimLoss.

Reference computation (see problem): for 8 RGB 1024x1024 image pairs,
convert to grayscale, tile into 256x256 tiles, then over a 9-level 2x2
sum-pooling pyramid accumulate  sum_d K[d] * (1 - mean(ssim_d))  with
ssim = (2st + C1) / (s^2 + t^2 + C1).

Key identities used here:
  * The 256x256 tiling is equivalent to hierarchical 2x2 pooling of the
    full 1024x1024 gray image (pool blocks never cross tile borders).
  * 1 - ssim = (s - t)^2 / (s^2 + t^2 + C1)  (exact), so each level only
    needs one ratio-sum.
  * The ratio is invariant under s,t -> lam*s, lam*t with C1 -> lam^2*C1.
    We compute gray/wb (wb = 0.114) so the grayscale conversion is two
    scalar_tensor_tensor ops per image, and use C1/wb^2 everywhere.

Sharding: pure data parallel - batch image b -> NeuronCore b.  Each core
computes level 8,7,6 ratio-sums on-device plus the pooled level-5 images;
the tiny tail (levels 5..0, 16K elems/image) and the final weighted mean
run on host in numpy during the gather step.
"""

import os
import sys

import numpy as np

for _p in ("/opt/trn_rl_repo",):
    if _p not in sys.path:
        sys.path.insert(0, _p)

import concourse.bacc as bacc
import concourse.bass as bass
import concourse.mybir as mybir
import concourse.tile as tile
from concourse.bass_utils import run_bass_kernel_spmd
from concourse.dve_ops import TENSOR_TENSOR_REDUCE


def _register_dve_ops():
    """Register two kernel-specific custom DVE ops (idempotent).

    DEN_SSIM:    out = in0^2 + in1^2 + s0            (the SSIM denominator)
    SQMUL_RED:   out = in0^2 * in1, accum = s0 + sum (ratio + reduction)

    The uops sha pins are computed here (same lower() that the table
    generator uses) instead of being hard-coded.
    """
    import concourse.dve_ops as dve_ops
    from concourse.dve_ops import DveOp
    from concourse.dve_spec import C0, Spec, Src0, Src1, _has_src1, lower, sq
    from concourse.dve_uop import DveOpSpec
    from operator import add as _add

    def _sha_for(name, spec):
        shas = {}
        for ver in ("v3",):
            row = dve_ops._SUB_OPCODE_FOR_NAME[name]
            s = DveOpSpec(
                name=name, opcode=row, uops=lower(spec, ver=ver),
                rd1_en=_has_src1(spec),
            )
            shas[ver] = s.sha(ver)
        return shas

    def _register(name, spec):
        if name in dve_ops._SUB_OPCODE_FOR_NAME:
            return next(op for op in dve_ops.OPS if op.name == name)
        row = dve_ops._CUSTOM_DVE_ROW_BASE + len(dve_ops.OPS)
        assert row < 0x20, "custom-DVE row field overflow"
        dve_ops._SUB_OPCODE_FOR_NAME[name] = row
        op = DveOp(name, spec, subdim=False, uops_sha=_sha_for(name, spec))
        dve_ops.OPS.append(op)
        dve_ops.CUSTOM_DVE_SPECS[name] = spec
        return op

    sqdiff_spec = Spec(
        body=sq(Src0 - Src1),
        reference=lambda in0, in1, s0, s1, imm2: (
            (in0.astype(np.float32) - in1.astype(np.float32)) ** 2
        ),
    )
    den_spec = Spec(
        body=sq(Src0) + sq(Src1) + C0,
        reference=lambda in0, in1, s0, s1, imm2: (
            in0.astype(np.float32) ** 2 + in1.astype(np.float32) ** 2 + s0
        ),
    )
    sqmul_spec = Spec(
        body=sq(Src0) * Src1,
        accum=_add,
        accum_init=C0,
        reference=dve_ops._ref_body_sum(
            lambda in0, in1, c0, c1, c2: in0.astype(np.float32) ** 2 * in1
        ),
    )
    return (
        _register("DEN_SSIM_ANT", den_spec),
        _register("SQMUL_RED_ANT", sqmul_spec),
        _register("SQDIFF_ANT", sqdiff_spec),
    )


DEN_SSIM, SQMUL_RED, SQDIFF = _register_dve_ops()

F32 = mybir.dt.float32
ALU = mybir.AluOpType
ACT = mybir.ActivationFunctionType

C1 = 0.2
WR, WG, WB = 0.299, 0.587, 0.114
C1T = C1 / (WG * WG)  # C1 for the (1/wg)-scaled gray values
K_LOSS = np.array([9, 8, 7, 6, 5, 4, 3, 2, 1], dtype=np.float64)  # K_LOSS[d]
N_CORES = 8
H = W = 1024

# acc columns: 8 for level-8 chunks, 4 for level-7, 2 for level-6
ACC_COLS = 14
L8_COLS = slice(0, 8)
L7_COLS = slice(8, 12)
L6_COLS = slice(12, 14)

LAST_RESULTS = None  # BassKernelResults of the most recent run (for profiling)

_CACHED_NC = None


def _ensure_ntff_hook():
    """Register the axon NTFF profile hook if the image's antenv lacks it.

    Only used when BASS_SSIM_TRACE=1 (profiling runs); the graded path
    never needs it.  Returns True when a usable hook is registered.
    """
    try:
        from antenv.axon_hooks import get_axon_ntff_profile_hook

        return get_axon_ntff_profile_hook() is not None
    except ImportError:
        pass
    try:
        import types

        import antenv
        from trn_agent_boot.trn_boot import _ntff_profile_via_ctypes

        mod = types.ModuleType("antenv.axon_hooks")
        _h = {}
        mod.set_axon_ntff_profile_hook = lambda h: _h.__setitem__("h", h)
        mod.get_axon_ntff_profile_hook = lambda: _h.get("h")
        sys.modules["antenv.axon_hooks"] = mod
        antenv.axon_hooks = mod
        hook = _ntff_profile_via_ctypes("/opt/axon/libaxon_pjrt.so")
        mod.set_axon_ntff_profile_hook(hook)
        # no artifact bucket in this container; keep files local
        from concourse import bass_utils as _bu

        _bu.upload_artifacts = lambda tmpdir: tmpdir
        return hook is not None
    except Exception as e:  # pragma: no cover - profiling-only path
        print(f"ntff hook setup failed: {type(e).__name__}: {e}")
        return False


def _pool_matrices():
    """Pa/Pb [128,128]: row-pair pooling of two stacked 128-row chunks.

    merged[j] = (Pa.T @ even_chunk + Pb.T @ odd_chunk)[j]
      j <  64: rows 2j, 2j+1 of the even chunk
      j >= 64: rows 2(j-64), 2(j-64)+1 of the odd chunk
    """
    pa = np.zeros((128, 128), dtype=np.float32)
    pb = np.zeros((128, 128), dtype=np.float32)
    for j in range(64):
        pa[2 * j, j] = 1.0
        pa[2 * j + 1, j] = 1.0
        pb[2 * j, 64 + j] = 1.0
        pb[2 * j + 1, 64 + j] = 1.0
    return pa, pb


def _build_nc():
    nc = bacc.Bacc("TRN2", target_bir_lowering=False, debug=False)

    inp = nc.declare_dram_parameter("input", [3, H, W], F32, isOutput=False)
    tgt = nc.declare_dram_parameter("target", [3, H, W], F32, isOutput=False)
    pa_d = nc.declare_dram_parameter("pa", [128, 128], F32, isOutput=False)
    pb_d = nc.declare_dram_parameter("pb", [128, 128], F32, isOutput=False)
    acc_d = nc.declare_dram_parameter("acc", [128, ACC_COLS], F32, isOutput=True)
    s5_d = nc.declare_dram_parameter("s5", [128, 128], F32, isOutput=True)
    t5_d = nc.declare_dram_parameter("t5", [128, 128], F32, isOutput=True)

    with tile.TileContext(nc) as tc:
        with (
            tc.tile_pool(name="singles", bufs=1) as singles,
            tc.tile_pool(name="rgb", bufs=2) as rgb_pool,
            tc.tile_pool(name="hbuf", bufs=3) as h_pool,
            tc.tile_pool(name="gray", bufs=6) as gray_pool,
            tc.tile_pool(name="tmp", bufs=2) as tmp_pool,
            tc.tile_pool(name="lvl", bufs=1) as lvl_pool,
            tc.tile_pool(name="psum", bufs=4, space="PSUM") as psum_pool,
        ):
            pa = singles.tile([128, 128], F32)
            pb = singles.tile([128, 128], F32)
            acc = singles.tile([128, ACC_COLS], F32)
            nc.sync.dma_start(pa[:], pa_d[:])
            nc.sync.dma_start(pb[:], pb_d[:])

            def gray_chunk(img_dram, j, tag):
                """Load RGB rows [128j:128j+128] and return wb-scaled gray."""
                r = rgb_pool.tile([128, W], F32, tag=f"r{tag}")
                g = rgb_pool.tile([128, W], F32, tag=f"g{tag}")
                b = rgb_pool.tile([128, W], F32, tag=f"b{tag}")
                rows = slice(128 * j, 128 * (j + 1))
                nc.sync.dma_start(r[:], img_dram[0, rows, :])
                nc.sync.dma_start(g[:], img_dram[1, rows, :])
                nc.sync.dma_start(b[:], img_dram[2, rows, :])
                h_t = h_pool.tile([128, W], F32, tag=f"h{tag}")
                # h = R*(wr/wg) + G
                nc.vector.scalar_tensor_tensor(
                    h_t[:], r[:], WR / WG, g[:], ALU.mult, ALU.add
                )
                m2 = h_pool.tile([128, W], F32, tag=f"m2{tag}")
                # m2 = B*(wb/wg)
                nc.scalar.activation(m2[:], b[:], ACT.Copy, scale=WB / WG)
                gray = gray_pool.tile([128, W], F32, tag=f"gray{tag}")
                # gray/wg = h + m2   (gpsimd only lowers plain tensor_tensor)
                nc.gpsimd.tensor_tensor(gray[:], h_t[:], m2[:], ALU.add)
                return gray

            def ssim_level(gx, gy, fd, acc_col):
                """acc[:, col] = per-partition sum of (gx-gy)^2/(gx^2+gy^2+C1T)."""
                diff = tmp_pool.tile([128, fd], F32, tag="diff")
                nc.vector.tensor_tensor(diff[:], gx[:], gy[:], ALU.subtract)
                den = tmp_pool.tile([128, fd], F32, tag="den")
                nc.vector._custom_dve(
                    DEN_SSIM, out=den[:], in0=gx[:], in1=gy[:], s0=C1T
                )
                rcp = tmp_pool.tile([128, fd], F32, tag="rcp")
                nc.vector.reciprocal_approx_fast(rcp[:], den[:])
                # out stream -> den (dead); accum_out is the partial sum
                nc.vector._custom_dve(
                    SQMUL_RED,
                    out=den[:],
                    in0=diff[:],
                    in1=rcp[:],
                    s0=0.0,
                    accum_out=acc[:, acc_col : acc_col + 1],
                )

            def pool_pair(src0, src1, fd, out_tag):
                """2x2 sum-pool two stacked [128, fd] chunks -> [128, fd//2]."""
                ps = psum_pool.tile([128, fd // 2, 2], F32)
                for h0 in range(0, fd, 512):
                    w = min(512, fd - h0)
                    out_ap = ps[:, h0 // 2 : (h0 + w) // 2, :]
                    nc.tensor.matmul(
                        out_ap, pa[:], src0[:, h0 : h0 + w], start=True, stop=False
                    )
                    nc.tensor.matmul(
                        out_ap, pb[:], src1[:, h0 : h0 + w], start=False, stop=True
                    )
                # PSUM->SBUF on scalar, column-pair add on gpsimd: keeps the
                # column pool entirely off the (bottleneck) vector engine
                cp = tmp_pool.tile([128, fd // 2, 2], F32, tag="cp")
                nc.scalar.activation(cp[:], ps[:], ACT.Copy)
                out = lvl_pool.tile([128, fd // 2], F32, tag=out_tag)
                nc.gpsimd.tensor_tensor(out[:], cp[:, :, 0], cp[:, :, 1], ALU.add)
                return out

            # ---- level 8 (8 chunks of [128, 1024]) + pool to level 7 ----
            s7, t7 = [], []
            for k in range(4):
                gxs, gys = [], []
                for j in (2 * k, 2 * k + 1):
                    gx = gray_chunk(inp, j, "x")
                    gy = gray_chunk(tgt, j, "y")
                    ssim_level(gx, gy, 1024, j)
                    gxs.append(gx)
                    gys.append(gy)
                s7.append(pool_pair(gxs[0], gxs[1], 1024, f"s7_{k}"))
                t7.append(pool_pair(gys[0], gys[1], 1024, f"t7_{k}"))

            # ---- level 7 (4 chunks of [128, 512]) + pool to level 6 ----
            s6, t6 = [], []
            for k in range(2):
                for j in (2 * k, 2 * k + 1):
                    ssim_level(s7[j], t7[j], 512, 8 + j)
                s6.append(pool_pair(s7[2 * k], s7[2 * k + 1], 512, f"s6_{k}"))
                t6.append(pool_pair(t7[2 * k], t7[2 * k + 1], 512, f"t6_{k}"))

            # ---- level 6 (2 chunks of [128, 256]) + pool to level 5 ----
            for j in (0, 1):
                ssim_level(s6[j], t6[j], 256, 12 + j)
            s5 = pool_pair(s6[0], s6[1], 256, "s5")
            t5 = pool_pair(t6[0], t6[1], 256, "t5")

            nc.sync.dma_start(acc_d[:], acc[:])
            nc.sync.dma_start(s5_d[:], s5[:])
            nc.sync.dma_start(t5_d[:], t5[:])

    nc.compile()
    return nc


def _get_nc():
    global _CACHED_NC
    if _CACHED_NC is None:
        _CACHED_NC = _build_nc()
    return _CACHED_NC


def _host_tail(per_core):
    """Combine per-core results into the scalar loss (float64 host math)."""
    total = 0.0
    # device levels: 8, 7, 6
    for d, cols in ((8, L8_COLS), (7, L7_COLS), (6, L6_COLS)):
        s = sum(float(r["acc"][:, cols].astype(np.float64).sum()) for r in per_core)
        cnt = N_CORES * 16 * 4**d
        total += K_LOSS[d] * (s / cnt)
    # host levels: 5..0 on the shipped pooled images (wb-scaled values)
    s = np.stack([r["s5"] for r in per_core]).astype(np.float64)
    t = np.stack([r["t5"] for r in per_core]).astype(np.float64)
    for d in range(5, -1, -1):
        ratio = (s - t) ** 2 / (s * s + t * t + C1T)
        cnt = N_CORES * 16 * 4**d
        total += K_LOSS[d] * (ratio.sum() / cnt)
        if d > 0:
            b, n, _ = s.shape
            s = s.reshape(b, n // 2, 2, n // 2, 2).sum(axis=(2, 4))
            t = t.reshape(b, n // 2, 2, n // 2, 2).sum(axis=(2, 4))
    return np.float32(total)


def kernel(input, target):
    global LAST_RESULTS
    input = np.ascontiguousarray(np.asarray(input, dtype=np.float32))
    target = np.ascontiguousarray(np.asarray(target, dtype=np.float32))
    assert input.shape == (N_CORES, 3, H, W), input.shape

    nc = _get_nc()
    pa, pb = _pool_matrices()
    in_maps = [
        {"input": input[i], "target": target[i], "pa": pa, "pb": pb}
        for i in range(N_CORES)
    ]
    trace = bool(int(os.environ.get("BASS_SSIM_TRACE", "0")))
    if trace:
        trace = _ensure_ntff_hook()
    res = run_bass_kernel_spmd(nc, in_maps, list(range(N_CORES)), trace=trace)
    LAST_RESULTS = res
    return _host_tail(res.results)



# revision 5
# speedup vs baseline: 1.4635x; 1.4635x over previous
"""Trainium2 Bass kernel for nn_DividedSsimLoss.

Reference: for 8 RGB 1024x1024 image pairs, grayscale, tile 256x256,
9-level 2x2 sum-pool pyramid, loss = sum_d K[d] * (1 - mean ssim_d),
ssim = (2st + C1) / (s^2 + t^2 + C1), i.e. 1-ssim = (s-t)^2/(s^2+t^2+C1).

This version (v2):
  * Inputs are shipped to the device as bf16, host-interleaved so each
    SBUF partition line is one 12 KiB contiguous DRAM span:
    rgbxy[chunk j, row p, channel c, x|y, col] -> [8, 128, 6144].
  * Grayscale runs on the tensor engine: 3 weighted-identity bf16
    matmuls (R,G,B) accumulate into PSUM; the scalar engine evacuates
    PSUM -> SBUF bf16.  gray is scaled by 1/wg, C1 -> C1/wg^2.
  * Per level the vector engine does 3 passes: bf16 subtract (2x mode),
    DEN = s^2+t^2+C1 (custom), and a fused RCPMUL custom op
    accum += (s-t)^2 * recip_approx(den)  (bitwise-not seed + 1 NR,
    ~0.4% worst-case; the loss is a mean of ~1M such terms).
  * 2x2 pooling: row pairs via Pa/Pb bf16 matmuls into PSUM, column
    pairs via one strided tensor_tensor add PSUM->SBUF (bf16 out).
  * Subtracts run on gpsimd; levels 7/6 are fused into single wide ops.

Sharding: pure data parallel, image b -> core b. Device computes levels
8,7,6 ratio-sums + pooled level-5 images; host does levels 5..0 in f64.
"""

import os
import sys

import numpy as np

for _p in ("/opt/trn_rl_repo",):
    if _p not in sys.path:
        sys.path.insert(0, _p)

import concourse.bacc as bacc
import concourse.bass as bass
import concourse.mybir as mybir
import concourse.tile as tile
from concourse.bass_utils import run_bass_kernel_spmd

from ml_dtypes import bfloat16 as np_bf16


def _register_dve_ops():
    """Register kernel-specific custom DVE ops (idempotent).

    DEN_SSIM:  out = in0^2 + in1^2 + s0
    RCPMUL:    out = in1^2 * y1(in0),  accum = sum(out)
               y1 = one-NR reciprocal approx of in0 (bitwise-not seed)
    """
    import concourse.dve_ops as dve_ops
    from concourse.dve_ops import DveOp
    from concourse.dve_spec import (
        C0,
        C1,
        AluOp,
        Bin,
        Spec,
        Src0,
        Src1,
        _has_src1,
        lower,
        sq,
    )
    from concourse.dve_uop import DveOpSpec
    from operator import add as _add

    def _sha_for(name, spec):
        shas = {}
        for ver in ("v3",):
            row = dve_ops._SUB_OPCODE_FOR_NAME[name]
            s = DveOpSpec(
                name=name, opcode=row, uops=lower(spec, ver=ver),
                rd1_en=_has_src1(spec),
            )
            shas[ver] = s.sha(ver)
        return shas

    def _register(name, spec):
        if name in dve_ops._SUB_OPCODE_FOR_NAME:
            return next(op for op in dve_ops.OPS if op.name == name)
        row = dve_ops._CUSTOM_DVE_ROW_BASE + len(dve_ops.OPS)
        assert row < 0x20, "custom-DVE row field overflow"
        dve_ops._SUB_OPCODE_FOR_NAME[name] = row
        op = DveOp(name, spec, subdim=False, uops_sha=_sha_for(name, spec))
        dve_ops.OPS.append(op)
        dve_ops.CUSTOM_DVE_SPECS[name] = spec
        return op

    den_spec = Spec(
        body=sq(Src0) + sq(Src1) + C0,
        reference=lambda in0, in1, s0, s1, imm2: (
            in0.astype(np.float32) ** 2 + in1.astype(np.float32) ** 2 + s0
        ),
    )

    # reciprocal seed: x * bitcast(~x) lands in [-4.5, -4]; one Chebyshev
    # scale + one NR pass (same constants as RECIPROCAL_APPROX_FAST).
    _nx = Bin(AluOp.BITWISE_NOT, Src0, Src0)
    _y0 = _nx * C0
    _y1 = _y0 * (C1 - Src0 * _y0)

    def _ref_rcpmul(in0, in1, c0, c1, c2):
        not_x = (~in0.astype(np.float32).view(np.int32)).view(np.float32)
        y0 = not_x * c0
        y1 = y0 * (c1 - in0.astype(np.float32) * y0)
        return in1.astype(np.float32) ** 2 * y1

    rcpmul_spec = Spec(
        body=sq(Src1) * _y1,
        accum=_add,
        reference=dve_ops._ref_body_sum(_ref_rcpmul),
    )

    return (
        _register("DEN_SSIM_ANT", den_spec),
        _register("RCPMUL_SSIM_ANT", rcpmul_spec),
    )


DEN_SSIM, RCPMUL = _register_dve_ops()

F32 = mybir.dt.float32
BF16 = mybir.dt.bfloat16
ALU = mybir.AluOpType
ACT = mybir.ActivationFunctionType

C1 = 0.2
WR, WG, WB = 0.299, 0.587, 0.114
C1T = C1 / (WG * WG)  # C1 for the (1/wg)-scaled gray values
RCP_C0 = -0.23549792
RCP_C1 = 2.0017324
K_LOSS = np.array([9, 8, 7, 6, 5, 4, 3, 2, 1], dtype=np.float64)  # K_LOSS[d]
N_CORES = 8
H = W = 1024

# acc columns: 8 for level-8 chunks, 1 for level-7, 1 for level-6
ACC_COLS = 10

LAST_RESULTS = None  # BassKernelResults of the most recent run (for profiling)

_CACHED_NC = None

SUB_ENGINE = os.environ.get("SSIM_SUB_ENGINE", "gpsimd")  # gpsimd | vector


def _ensure_ntff_hook():
    """Register the axon NTFF profile hook if the image's antenv lacks it."""
    try:
        from antenv.axon_hooks import get_axon_ntff_profile_hook

        return get_axon_ntff_profile_hook() is not None
    except ImportError:
        pass
    try:
        import types

        import antenv
        from trn_agent_boot.trn_boot import _ntff_profile_via_ctypes

        mod = types.ModuleType("antenv.axon_hooks")
        _h = {}
        mod.set_axon_ntff_profile_hook = lambda h: _h.__setitem__("h", h)
        mod.get_axon_ntff_profile_hook = lambda: _h.get("h")
        sys.modules["antenv.axon_hooks"] = mod
        antenv.axon_hooks = mod
        hook = _ntff_profile_via_ctypes("/opt/axon/libaxon_pjrt.so")
        mod.set_axon_ntff_profile_hook(hook)
        from concourse import bass_utils as _bu

        _bu.upload_artifacts = lambda tmpdir: tmpdir
        return hook is not None
    except Exception as e:  # pragma: no cover - profiling-only path
        print(f"ntff hook setup failed: {type(e).__name__}: {e}")
        return False


def _weight_matrices():
    """[5,128,128] bf16: wr*I, wg*I, wb*I (scaled 1/wg), Pa, Pb."""
    w = np.zeros((5, 128, 128), dtype=np.float32)
    w[0] = np.eye(128, dtype=np.float32) * (WR / WG)
    w[1] = np.eye(128, dtype=np.float32)
    w[2] = np.eye(128, dtype=np.float32) * (WB / WG)
    for j in range(64):
        w[3, 2 * j, j] = 1.0       # Pa: row pairs of even chunk -> part 0..63
        w[3, 2 * j + 1, j] = 1.0
        w[4, 2 * j, 64 + j] = 1.0  # Pb: row pairs of odd chunk -> part 64..127
        w[4, 2 * j + 1, 64 + j] = 1.0
    return w.astype(np_bf16)


def _build_nc():
    nc = bacc.Bacc("TRN2", target_bir_lowering=False, debug=False)

    rgb_d = nc.declare_dram_parameter("rgbxy", [8, 128, 6144], BF16, isOutput=False)
    wts_d = nc.declare_dram_parameter("wts", [5, 128, 128], BF16, isOutput=False)
    acc_d = nc.declare_dram_parameter("acc", [128, ACC_COLS], F32, isOutput=True)
    s5t5_d = nc.declare_dram_parameter("s5t5", [128, 256], F32, isOutput=True)

    with tile.TileContext(nc) as tc:
        with (
            tc.tile_pool(name="singles", bufs=1) as singles,
            tc.tile_pool(name="rgb", bufs=4) as rgb_pool,
            tc.tile_pool(name="gray", bufs=4) as gray_pool,
            tc.tile_pool(name="sd", bufs=2) as sd_pool,
            tc.tile_pool(name="pg", bufs=2, space="PSUM") as pg_pool,
            tc.tile_pool(name="pp", bufs=2, space="PSUM") as pp_pool,
        ):
            wt = [
                singles.tile([128, 128], BF16, tag=f"w{i}", name=f"w{i}")
                for i in range(5)
            ]
            for i in range(5):
                nc.sync.dma_start(wt[i][:], wts_d[i])
            wr_t, wg_t, wb_t, pa, pb = wt

            acc = singles.tile([128, ACC_COLS], F32)
            s5t5 = singles.tile([128, 256], F32)
            s7all = singles.tile([128, 2048], BF16, tag="s7all")
            t7all = singles.tile([128, 2048], BF16, tag="t7all")
            s6all = singles.tile([128, 512], BF16, tag="s6all")
            t6all = singles.tile([128, 512], BF16, tag="t6all")
            dead = singles.tile([128, 2048], BF16, tag="dead")

            rgb = [
                rgb_pool.tile([128, 6144], BF16, tag="rgb", name=f"rgb{j}")
                for j in range(8)
            ]
            for j in range(8):
                nc.sync.dma_start(rgb[j][:], rgb_d[j])

            def gray_chunk(j):
                """PE: gray/wg of chunk j -> [128, 2048] bf16 (x | y)."""
                psA = pg_pool.tile([128, 1024], F32, tag="pg")
                psB = pg_pool.tile([128, 1024], F32, tag="pg")
                for ci, wm in enumerate((wr_t, wg_t, wb_t)):
                    start = ci == 0
                    stop = ci == 2
                    for t in range(4):
                        out = (
                            psA[:, 512 * t : 512 * (t + 1)]
                            if t < 2
                            else psB[:, 512 * (t - 2) : 512 * (t - 1)]
                        )
                        nc.tensor.matmul(
                            out,
                            wm[:],
                            rgb[j][:, 2048 * ci + 512 * t : 2048 * ci + 512 * (t + 1)],
                            start=start,
                            stop=stop,
                        )
                g = gray_pool.tile([128, 2048], BF16, tag="g8")
                nc.scalar.activation(g[:, 0:1024], psA[:], ACT.Copy)
                nc.scalar.activation(g[:, 1024:2048], psB[:], ACT.Copy)
                return g

            def ssim(gx_ap, gy_ap, fd, col, tag):
                """acc[:, col] = per-partition sum of (s-t)^2/(s^2+t^2+C1T)."""
                diff = sd_pool.tile([128, fd], BF16, tag=f"d{tag}")
                eng = nc.gpsimd if SUB_ENGINE == "gpsimd" else nc.vector
                eng.tensor_tensor(diff[:], gx_ap, gy_ap, ALU.subtract)
                den = sd_pool.tile([128, fd], BF16, tag=f"n{tag}")
                nc.vector._custom_dve(
                    DEN_SSIM, out=den[:], in0=gx_ap, in1=gy_ap, s0=C1T
                )
                nc.vector._custom_dve(
                    RCPMUL,
                    out=dead[:, 0:fd],
                    in0=den[:],
                    in1=diff[:],
                    s0=RCP_C0,
                    s1=RCP_C1,
                    accum_out=acc[:, col : col + 1],
                )

            def pool_pair(even_ap, odd_ap, fd, out_ap):
                """2x2 sum-pool two stacked [128, fd] chunks -> out_ap [128, fd//2].

                Row pairs via Pa/Pb, column pairs via stride-2 rhs views —
                all four contributions accumulate in one PSUM region, then
                the scalar engine evacuates it."""
                ps = pp_pool.tile([128, 512], F32, tag="pp", name="ps")
                half = fd // 2
                seq = (
                    (pa, even_ap[:, 0:fd:2]),
                    (pa, even_ap[:, 1:fd:2]),
                    (pb, odd_ap[:, 0:fd:2]),
                    (pb, odd_ap[:, 1:fd:2]),
                )
                for i, (wm, src) in enumerate(seq):
                    nc.tensor.matmul(
                        ps[:, 0:half], wm[:], src, start=(i == 0), stop=(i == 3)
                    )
                nc.scalar.activation(out_ap, ps[:, 0:half], ACT.Copy)

            # ---- level 8: gray, ssim, pool to level 7 ----
            g8 = [None] * 8
            for j in range(8):
                g8[j] = gray_chunk(j)
                if j % 2 == 1:
                    k = j // 2
                    ge, go = g8[j - 1], g8[j]
                    ssim(ge[:, 0:1024], ge[:, 1024:2048], 1024, j - 1, "8")
                    ssim(go[:, 0:1024], go[:, 1024:2048], 1024, j, "8")
                    pool_pair(
                        ge[:, 0:1024], go[:, 0:1024], 1024,
                        s7all[:, 512 * k : 512 * (k + 1)],
                    )
                    pool_pair(
                        ge[:, 1024:2048], go[:, 1024:2048], 1024,
                        t7all[:, 512 * k : 512 * (k + 1)],
                    )

            # ---- level 7 (one fused [128, 2048] pass) + pool to level 6 ----
            ssim(s7all[:], t7all[:], 2048, 8, "7")
            for k in range(2):
                pool_pair(
                    s7all[:, 1024 * k : 1024 * k + 512],
                    s7all[:, 1024 * k + 512 : 1024 * (k + 1)],
                    512,
                    s6all[:, 256 * k : 256 * (k + 1)],
                )
                pool_pair(
                    t7all[:, 1024 * k : 1024 * k + 512],
                    t7all[:, 1024 * k + 512 : 1024 * (k + 1)],
                    512,
                    t6all[:, 256 * k : 256 * (k + 1)],
                )

            # ---- level 6 (one fused [128, 512] pass) + pool to level 5 ----
            ssim(s6all[:], t6all[:], 512, 9, "6")
            pool_pair(s6all[:, 0:256], s6all[:, 256:512], 256, s5t5[:, 0:128])
            pool_pair(t6all[:, 0:256], t6all[:, 256:512], 256, s5t5[:, 128:256])

            nc.sync.dma_start(acc_d[:], acc[:])
            nc.sync.dma_start(s5t5_d[:], s5t5[:])

    nc.compile()
    return nc


def _get_nc():
    global _CACHED_NC
    if _CACHED_NC is None:
        _CACHED_NC = _build_nc()
    return _CACHED_NC


def _host_tail(per_core):
    """Combine per-core results into the scalar loss (float64 host math)."""
    total = 0.0
    # device levels: 8 (acc cols 0..7), 7 (col 8), 6 (col 9)
    for d, cols in ((8, slice(0, 8)), (7, slice(8, 9)), (6, slice(9, 10))):
        s = sum(float(r["acc"][:, cols].astype(np.float64).sum()) for r in per_core)
        cnt = N_CORES * 16 * 4**d
        total += K_LOSS[d] * (s / cnt)
    # host levels: 5..0 on the shipped pooled images ((1/wg)-scaled values)
    s = np.stack([r["s5t5"][:, 0:128] for r in per_core]).astype(np.float64)
    t = np.stack([r["s5t5"][:, 128:256] for r in per_core]).astype(np.float64)
    for d in range(5, -1, -1):
        ratio = (s - t) ** 2 / (s * s + t * t + C1T)
        cnt = N_CORES * 16 * 4**d
        total += K_LOSS[d] * (ratio.sum() / cnt)
        if d > 0:
            b, n, _ = s.shape
            s = s.reshape(b, n // 2, 2, n // 2, 2).sum(axis=(2, 4))
            t = t.reshape(b, n // 2, 2, n // 2, 2).sum(axis=(2, 4))
    return np.float32(total)


def _pack_inputs(input, target):
    """[8,3,1024,1024] f32 x2 -> per-core [8,128,6144] bf16, interleaved
    so each partition line is [c, x|y, 1024] = 12 KiB contiguous."""
    out = np.empty((N_CORES, 8, 128, 3, 2, 1024), dtype=np_bf16)
    out[:, :, :, :, 0, :] = input.reshape(8, 3, 8, 128, 1024).transpose(0, 2, 3, 1, 4)
    out[:, :, :, :, 1, :] = target.reshape(8, 3, 8, 128, 1024).transpose(0, 2, 3, 1, 4)
    return out.reshape(N_CORES, 8, 128, 6144)


def kernel(input, target):
    global LAST_RESULTS
    input = np.ascontiguousarray(np.asarray(input, dtype=np.float32))
    target = np.ascontiguousarray(np.asarray(target, dtype=np.float32))
    assert input.shape == (N_CORES, 3, H, W), input.shape

    nc = _get_nc()
    rgbxy = _pack_inputs(input, target)
    wts = _weight_matrices()
    in_maps = [{"rgbxy": rgbxy[i], "wts": wts} for i in range(N_CORES)]
    trace = bool(int(os.environ.get("BASS_SSIM_TRACE", "0")))
    if trace:
        trace = _ensure_ntff_hook()
    res = run_bass_kernel_spmd(nc, in_maps, list(range(N_CORES)), trace=trace)
    LAST_RESULTS = res
    return _host_tail(res.results)
